# revision 23
# baseline (speedup 1.0000x reference)
"""Trainium2 Bass kernel for nn_BERTRegression_72945724555435.

Reference computation (B=32, T=4096, H=256):
    pen[b,t]  = (1 - mask[b,t]) * 1e6
    xm        = x - pen[...,None]
    w[t]      = EMA weights (alpha=0.1, closed form)
    ema[b,h]  = sum_t w[t] * xm[b,t,h]
    mean[b,h] = sum_t xm[b,t,h] / T
    pooled    = weight_ema * ema + weight_mean * mean
    out[b]    = pooled @ W.T + bias

Algebraic reduction (exact in real arithmetic):
    c[t]   = weight_ema * w[t] + weight_mean / T
    y[b,h] = sum_t c[t] * x[b,t,h]                  (the only large compute)
    q[b]   = sum_t (1e6 * Wsum * c[t]) * mask[b,t]
    out[b] = sum_h W[h] * y[b,h] + q[b] + (bias - 1e6 * Wsum * sum_t c[t])

Data-parallel over batch: 8 cores x 4 samples. The kernel is HBM-bandwidth
bound, so x is streamed as fp8 (e4m3): 4 MiB/core, 4x less traffic than
f32. The weight vector c is scaled by a power of two S into fp8's normal
range; S is divided back out of the final [1,H] linear (applied in f32).
The mask penalty path -- which dominates the output magnitude -- stays in
f32/int on device.

PE: plain fp8 matmuls, 4-way column tiling. Chunk 4j+g goes to column
group g (tile_position=(0,32g)); the four groups stream their moving
operands on separate XBUSes concurrently, lifting PE ingest to
4x128 elem/cycle so DMA remains the only bottleneck. The c operand is
replicated across 32 stationary columns per group so every PSUM partition
in ys[b] [128,H] is written (rows within a group all equal y_g). Finals:
per-sample DVE mul-by-W + row reduce into zall[:,b], then one PE pass
out[b] = sum_p zall[p,b]*rowsel[p] + sum_p sel[p,b]*mq2[p] folds the
column-group sum and the mask path in a single PSUM accumulation.
"""

import numpy as np

N_CORES = 8
B, T, H = 32, 4096, 256
BS = B // N_CORES          # samples per core
NK = T // 128              # 128-row t-chunks per sample (32)
NKT = NK // 2              # chunks per half-sample tile (16)
NTILE = BS * 2             # x tiles per core body (half-sample each)
NGRP = 4                   # PE column groups
MREP = 32                  # replicated stationary columns per group
ALPHA = 0.1
PEN = 1.0e6

_PROGRAM_CACHE = {}


def _build_program(repeats=1, hw_loop=0):
    """Build the Bass program (one NeuronCore's view: BS samples).

    hw_loop=n means n total bodies (For_i(n//UNROLL) x UNROLL)."""
    import concourse.bass as bass
    import concourse.tile as tile
    from concourse import mybir

    f32 = mybir.dt.float32
    f8 = mybir.dt.float8e4
    u8 = mybir.dt.uint8

    def _legalize_waits(nc):
        """The walrus build in this container accepts at most one sync wait
        per instruction (two on EventSemaphore), but Tile emits more. Split
        the excess waits onto same-engine NOPs inserted right before the
        offending instruction -- per-engine program order makes this
        semantically identical."""
        for bb in nc.m.functions[0].blocks:
            new_insts = []
            for inst in bb.instructions:
                si = getattr(inst, "sync_info", None)
                cap = 2 if isinstance(inst, mybir.InstEventSemaphore) else 1
                if si is not None and len(si.on_wait) > cap:
                    waits = list(si.on_wait)
                    for j, w in enumerate(waits[: -cap]):
                        nop = mybir.InstNoOp(
                            name=f"{inst.name}-ws{j}",
                            engine=inst.engine,
                            bass_nofuse=True,
                            sync_info=mybir.SyncInfo(on_wait=[w], on_update=[]),
                        )
                        nc.register_instruction(nop)
                        new_insts.append(nop)
                    si.on_wait = waits[-cap:]
                new_insts.append(inst)
            bb.instructions[:] = new_insts

    nc = bass.Bass("TRN2", target_bir_lowering=False, debug=False)

    x_ap = nc.dram_tensor("x", [NTILE, 128, NKT, H], f8, kind="ExternalInput").ap()
    mask_ap = nc.dram_tensor("mask", [128, 128], u8, kind="ExternalInput").ap()
    ccols_ap = nc.dram_tensor("ccols", [128, NK, MREP], f8, kind="ExternalInput").ap()
    c2g_ap = nc.dram_tensor("c2grid", [128, 128], f32, kind="ExternalInput").ap()
    sel_ap = nc.dram_tensor("sel", [128, BS], f32, kind="ExternalInput").ap()
    w_ap = nc.dram_tensor("w", [128, H], f32, kind="ExternalInput").ap()
    k0_ap = nc.dram_tensor("k0", [128, 1], f32, kind="ExternalInput").ap()
    rsel_ap = nc.dram_tensor("rowsel", [128, 1], mybir.dt.float32r, kind="ExternalInput").ap()
    out_ap = nc.dram_tensor("out", [1, BS], f32, kind="ExternalOutput").ap()

    with tile.TileContext(nc) as tc:
        with (
            tc.tile_pool(name="const", bufs=1) as cpool,
            tc.tile_pool(name="xp", bufs=12) as xpool,
            tc.tile_pool(name="small", bufs=2) as spool,
            tc.tile_pool(name="psum", bufs=1, space="PSUM") as ppool,
        ):
            ccols = cpool.tile([128, NK, MREP], f8)
            nc.gpsimd.dma_start(ccols[:], ccols_ap[:])
            c2g = cpool.tile([128, 128], f32)
            nc.gpsimd.dma_start(c2g[:], c2g_ap[:])
            sel = cpool.tile([128, BS], f32)
            nc.gpsimd.dma_start(sel[:], sel_ap[:])
            wsb = cpool.tile([128, H], f32)
            nc.gpsimd.dma_start(wsb[:], w_ap[:])
            k0sb = cpool.tile([128, 1], f32)
            nc.gpsimd.dma_start(k0sb[:], k0_ap[:])
            rsel = cpool.tile([128, 1], mybir.dt.float32r)
            nc.gpsimd.dma_start(rsel[:], rsel_ap[:])
            mtile = cpool.tile([128, 128], u8)
            nc.gpsimd.dma_start(mtile[:], mask_ap[:])

            def emit_body(rep, upar=0):
                # mask path: mq2[p] = K0/128 + sum_f mask[p,f]*c2grid[p,f]
                maskf = spool.tile([128, 128], f32, tag="maskf", name=f"maskf{rep}")
                nc.vector.tensor_copy(maskf[:], mtile[:])
                nc.vector.tensor_mul(maskf[:], maskf[:], c2g[:])
                mq = spool.tile([128, 1], f32, tag="mq", name=f"mq{rep}")
                nc.vector.reduce_sum(mq[:], maskf[:], axis=mybir.AxisListType.X)
                mq2 = spool.tile([128, 1], f32, tag="mq2", name=f"mq2{rep}")
                nc.vector.tensor_scalar_add(mq2[:], mq[:], k0sb[:])
                q_ps = ppool.tile([1, BS], f32, tag="q", name=f"q{rep}")
                nc.tensor.matmul(
                    q_ps[:], lhsT=mq2[:], rhs=sel[:], start=True, stop=True
                )
                q_sb = spool.tile([1, BS], f32, tag="qsb", name=f"qsb{rep}")
                nc.vector.tensor_copy(q_sb[:], q_ps[:])
                # main path: ys[b][32g+m, h] accumulates y_g = the partial
                # dot over chunks 4j+g; 4 col groups stream concurrently.
                ys = [
                    ppool.tile([128, H], f32, tag=f"y{b}", name=f"y{b}_{rep}")
                    for b in range(BS)
                ]
                zall = spool.tile(
                    [128, BS], mybir.dt.float32r, tag="zall", name=f"zall{rep}"
                )
                for b in range(BS):
                    for half in range(2):
                        i = b * 2 + half
                        xt = xpool.tile(
                            [128, NKT, H], f8, tag="xt", name=f"xt{rep}_{i}"
                        )
                        qs = [nc.sync, nc.scalar]
                        if i == NTILE - 1:
                            # last tile: wave-aligned pieces, thinning to a
                            # single final chunk so only 1 matmul + the short
                            # final chain depend on the last transfer
                            cuts = [0, 4, 8, 12, 15, 16]
                            for q in range(len(cuts) - 1):
                                lo, hi = cuts[q], cuts[q + 1]
                                qs[(upar + q) % 2].dma_start(
                                    xt[:, lo:hi, :], x_ap[i][:, lo:hi, :]
                                )
                        else:
                            # queue parity tied to body parity: the queue that
                            # stores fin in body u never carries body u+1's
                            # first tile, so the fin-wait can't delay it
                            qs[(upar + i) % 2].dma_start(xt[:], x_ap[i])
                        for jj in range(NKT // NGRP):
                            j = half * (NKT // NGRP) + jj
                            for g in range(NGRP):
                                gk = NGRP * j + g
                                nc.tensor.matmul(
                                    ys[b][32 * g : 32 * g + MREP, :],
                                    lhsT=ccols[:, gk : gk + 1, :],
                                    rhs=xt[:, NGRP * jj + g, :],
                                    start=(j == 0),
                                    stop=(j == NK // NGRP - 1),
                                    tile_position=(0, 32 * g),
                                )
                    # z[p] = sum_h ys[b][p,h] * W[h]/S  (per-sample, overlaps
                    # the next sample's matmuls)
                    tmp = spool.tile([128, H], f32, tag="tmp", name=f"tmp{rep}_{b}")
                    nc.vector.tensor_mul(tmp[:], ys[b][:], wsb[:])
                    with nc.allow_low_precision("f32r y-path dot, ~1e-5 rel"):
                        nc.vector.reduce_sum(
                            zall[:, b : b + 1], tmp[:], axis=mybir.AxisListType.X
                        )

                # out[b] = sum_p zall[p,b]*rowsel[p] + sum_p sel[p,b]*mq2[p]
                o_ps = ppool.tile([1, BS], f32, tag="o", name=f"o{rep}")
                nc.tensor.matmul(
                    o_ps[:], lhsT=rsel[:], rhs=zall[:], start=True, stop=True
                )
                fin = spool.tile([1, BS], f32, tag="fin", name=f"fin{rep}")
                nc.vector.tensor_add(fin[:], o_ps[:], q_sb[:])
                [nc.sync, nc.scalar][upar % 2].dma_start(out_ap[:], fin[:])

            if hw_loop:
                unroll = 4 if hw_loop % 4 == 0 else (2 if hw_loop % 2 == 0 else 1)
                with tc.For_i(0, hw_loop // unroll):
                    for u in range(unroll):
                        emit_body(u, u)
            else:
                for rep in range(repeats):
                    emit_body(rep)

    _legalize_waits(nc)
    return nc


def _prepare_in_maps(x, mask, weight_ema, weight_mean, W, b):
    """Host-side prep: fold the tiny scalar weights into the c vectors
    (float64), quantize x and the scaled c to fp8, shard x/mask over the
    batch dim."""
    import ml_dtypes

    f8 = ml_dtypes.float8_e4m3

    x = np.asarray(x, dtype=np.float32)
    mask = np.asarray(mask)
    weight_ema = np.asarray(weight_ema, dtype=np.float64)
    weight_mean = np.asarray(weight_mean, dtype=np.float64)
    W = np.asarray(W, dtype=np.float64)
    b = np.asarray(b, dtype=np.float64)

    pows = (1.0 - ALPHA) ** np.arange(T - 1, -1, -1, dtype=np.float64)
    wv = ALPHA * pows
    wv[0] = pows[0]
    c = np.float64(weight_ema[0]) * wv + np.float64(weight_mean[0]) / T
    Wsum = float(W.sum())
    c2 = PEN * Wsum * c
    K0 = float(b[0]) - PEN * Wsum * float(c.sum())

    # power-of-two scale putting max|c| ~ 64, well inside fp8e4 normals
    cmax = float(np.abs(c).max())
    S = float(2.0 ** np.floor(np.log2(64.0 / cmax))) if cmax > 0 else 1.0

    # ccols[p, k, m] = S * c[k*128 + p] for every replicated column m
    cq = (c * S).reshape(NK, 128).T.astype(f8)
    ccols = np.ascontiguousarray(np.repeat(cq[:, :, None], MREP, axis=2))
    # c2grid[p, f] = c2[(p % 32) * 128 + f]  (matches mask.reshape(128,128))
    c2grid = np.ascontiguousarray(
        np.tile(c2.reshape(T // 128, 128), (BS, 1)), dtype=np.float32
    )
    sel = np.zeros((128, BS), dtype=np.float32)
    for bb in range(BS):
        sel[bb * (128 // BS) : (bb + 1) * (128 // BS), bb] = 1.0
    w_in = np.ascontiguousarray(
        np.broadcast_to(W.reshape(1, H) / S, (128, H)), dtype=np.float32
    )
    # sel has 32 ones per sample, so K0/32 per partition sums back to K0
    k0_in = np.full((128, 1), K0 / (128 // BS), dtype=np.float32)
    # rowsel: 1.0 at one representative row per column group
    rowsel = np.zeros((128, 1), dtype=np.float32)
    rowsel[[0, 32, 64, 96], 0] = 1.0

    # x tile layout: [b, half, p, k, h] with t = (half*NKT + k)*128 + p
    x8 = x.astype(f8).reshape(B // BS, BS, 2, NKT, 128, H)
    in_maps = []
    for i in range(N_CORES):
        xs = np.ascontiguousarray(x8[i].transpose(0, 1, 3, 2, 4)).reshape(
            NTILE, 128, NKT, H
        )
        ms = np.ascontiguousarray(
            mask[i * BS : (i + 1) * BS].reshape(128, 128).astype(np.uint8)
        )
        in_maps.append(
            {
                "x": xs,
                "mask": ms,
                "ccols": ccols,
                "c2grid": c2grid,
                "sel": sel,
                "w": w_in,
                "k0": k0_in,
                "rowsel": rowsel,
            }
        )
    return in_maps


def _run(inputs, trace=False):
    from concourse.bass_utils import run_bass_kernel_spmd

    if "nc" not in _PROGRAM_CACHE:
        _PROGRAM_CACHE["nc"] = _build_program(repeats=1)
    nc = _PROGRAM_CACHE["nc"]
    in_maps = _prepare_in_maps(**inputs)
    res = run_bass_kernel_spmd(nc, in_maps, list(range(N_CORES)), trace=trace)
    out = np.concatenate(
        [res.results[i]["out"].reshape(BS) for i in range(N_CORES)]
    ).astype(np.float32)
    return out, res


def kernel(**inputs) -> np.ndarray:
    out, _ = _run(inputs, trace=False)
    return out


# revision 24
# speedup vs baseline: 1.0172x; 1.0172x over previous
"""Trainium2 Bass kernel for nn_BERTRegression_72945724555435.

Reference computation (B=32, T=4096, H=256):
    pen[b,t]  = (1 - mask[b,t]) * 1e6
    xm        = x - pen[...,None]
    w[t]      = EMA weights (alpha=0.1, closed form)
    ema[b,h]  = sum_t w[t] * xm[b,t,h]
    mean[b,h] = sum_t xm[b,t,h] / T
    pooled    = weight_ema * ema + weight_mean * mean
    out[b]    = pooled @ W.T + bias

Algebraic reduction (exact in real arithmetic):
    c[t]   = weight_ema * w[t] + weight_mean / T
    y[b,h] = sum_t c[t] * x[b,t,h]                  (the only large compute)
    q[b]   = sum_t (1e6 * Wsum * c[t]) * mask[b,t]
    out[b] = sum_h W[h] * y[b,h] + q[b] + (bias - 1e6 * Wsum * sum_t c[t])

Data-parallel over batch: 8 cores x 4 samples. The kernel is HBM-bandwidth
bound, so x is streamed as fp8 (e4m3): 4 MiB/core, 4x less traffic than
f32. The weight vector c is scaled by a power of two S into fp8's normal
range; S is divided back out of the final [1,H] linear (applied in f32).
The mask penalty path -- which dominates the output magnitude -- stays in
f32/int on device.

PE: plain fp8 matmuls, 4-way column tiling. Chunk 4j+g goes to column
group g (tile_position=(0,32g)); the four groups stream their moving
operands on separate XBUSes concurrently, lifting PE ingest above the
1x128 elem/cycle single-stream rate (11.3us PE-only vs 13.2us DMA-only),
so DMA remains the only bottleneck. The c operand is replicated across
32 stationary columns per group so every PSUM partition in ys[b] [128,H]
is written (rows within a group all equal y_g).

Overlap structure (tuned against hw-loop slope measurements; DMA-only
replica of this structure measures 13.2us, full body ~13.8us):
- x streams as 512 KiB half-sample tiles alternating between the two
  HWDGE queues (SP/ACT); the last tile is wave-aligned quarters so only
  4 matmuls + the short final chain depend on the final transfer.
- 12-deep tile pool lets DMA run more than a full body ahead.
- The mask-path matmul (true f32, 4 PE passes) runs at body start into
  its own PSUM tile; the body tail is one single-pass f32r matmul
  (rowsel . zall), a DVE add with the early mask result, and the store.
- The timing hw-loop unrolls 4 bodies per For_i iteration; queue parity
  rotates per body so the queue that waits on fin to store it never
  carries the next body's first tile.
"""

import numpy as np

N_CORES = 8
B, T, H = 32, 4096, 256
BS = B // N_CORES          # samples per core
NK = T // 128              # 128-row t-chunks per sample (32)
NKT = NK // 2              # chunks per half-sample tile (16)
NTILE = BS * 2             # x tiles per core body (half-sample each)
NGRP = 4                   # PE column groups
MREP = 32                  # replicated stationary columns per group
ALPHA = 0.1
PEN = 1.0e6

_PROGRAM_CACHE = {}


def _build_program(repeats=1, hw_loop=0):
    """Build the Bass program (one NeuronCore's view: BS samples).

    hw_loop=n means n total bodies (For_i(n//UNROLL) x UNROLL)."""
    import concourse.bass as bass
    import concourse.tile as tile
    from concourse import mybir

    f32 = mybir.dt.float32
    f8 = mybir.dt.float8e4
    u8 = mybir.dt.uint8

    def _legalize_waits(nc):
        """The walrus build in this container accepts at most one sync wait
        per instruction (two on EventSemaphore), but Tile emits more. Split
        the excess waits onto same-engine NOPs inserted right before the
        offending instruction -- per-engine program order makes this
        semantically identical."""
        for bb in nc.m.functions[0].blocks:
            new_insts = []
            for inst in bb.instructions:
                si = getattr(inst, "sync_info", None)
                cap = 2 if isinstance(inst, mybir.InstEventSemaphore) else 1
                if si is not None and len(si.on_wait) > cap:
                    waits = list(si.on_wait)
                    for j, w in enumerate(waits[: -cap]):
                        nop = mybir.InstNoOp(
                            name=f"{inst.name}-ws{j}",
                            engine=inst.engine,
                            bass_nofuse=True,
                            sync_info=mybir.SyncInfo(on_wait=[w], on_update=[]),
                        )
                        nc.register_instruction(nop)
                        new_insts.append(nop)
                    si.on_wait = waits[-cap:]
                new_insts.append(inst)
            bb.instructions[:] = new_insts

    nc = bass.Bass("TRN2", target_bir_lowering=False, debug=False)

    x_ap = nc.dram_tensor("x", [NTILE, 128, NKT, H], f8, kind="ExternalInput").ap()
    mask_ap = nc.dram_tensor("mask", [128, 128], u8, kind="ExternalInput").ap()
    ccols_ap = nc.dram_tensor("ccols", [128, NK, MREP], f8, kind="ExternalInput").ap()
    c2g_ap = nc.dram_tensor("c2grid", [128, 128], f32, kind="ExternalInput").ap()
    sel_ap = nc.dram_tensor("sel", [128, BS], f32, kind="ExternalInput").ap()
    w_ap = nc.dram_tensor("w", [128, H], f32, kind="ExternalInput").ap()
    k0_ap = nc.dram_tensor("k0", [128, 1], f32, kind="ExternalInput").ap()
    rsel_ap = nc.dram_tensor("rowsel", [128, 1], mybir.dt.float32r, kind="ExternalInput").ap()
    out_ap = nc.dram_tensor("out", [1, BS], f32, kind="ExternalOutput").ap()

    with tile.TileContext(nc) as tc:
        with (
            tc.tile_pool(name="const", bufs=1) as cpool,
            tc.tile_pool(name="xp", bufs=12) as xpool,
            tc.tile_pool(name="small", bufs=2) as spool,
            tc.tile_pool(name="psum", bufs=1, space="PSUM") as ppool,
        ):
            ccols = cpool.tile([128, NK, MREP], f8)
            nc.gpsimd.dma_start(ccols[:], ccols_ap[:])
            c2g = cpool.tile([128, 128], f32)
            nc.gpsimd.dma_start(c2g[:], c2g_ap[:])
            sel = cpool.tile([128, BS], f32)
            nc.gpsimd.dma_start(sel[:], sel_ap[:])
            wsb = cpool.tile([128, H], f32)
            nc.gpsimd.dma_start(wsb[:], w_ap[:])
            k0sb = cpool.tile([128, 1], f32)
            nc.gpsimd.dma_start(k0sb[:], k0_ap[:])
            rsel = cpool.tile([128, 1], mybir.dt.float32r)
            nc.gpsimd.dma_start(rsel[:], rsel_ap[:])
            mtile = cpool.tile([128, 128], u8)
            nc.gpsimd.dma_start(mtile[:], mask_ap[:])

            def emit_body(rep, upar=0):
                # mask path: mq2[p] = K0/128 + sum_f mask[p,f]*c2grid[p,f]
                maskf = spool.tile([128, 128], f32, tag="maskf", name=f"maskf{rep}")
                nc.vector.tensor_copy(maskf[:], mtile[:])
                nc.vector.tensor_mul(maskf[:], maskf[:], c2g[:])
                mq = spool.tile([128, 1], f32, tag="mq", name=f"mq{rep}")
                nc.vector.reduce_sum(mq[:], maskf[:], axis=mybir.AxisListType.X)
                mq2 = spool.tile([128, 1], f32, tag="mq2", name=f"mq2{rep}")
                nc.vector.tensor_scalar_add(mq2[:], mq[:], k0sb[:])
                q_ps = ppool.tile([1, BS], f32, tag="q", name=f"q{rep}")
                nc.tensor.matmul(
                    q_ps[:], lhsT=mq2[:], rhs=sel[:], start=True, stop=True
                )
                q_sb = spool.tile([1, BS], f32, tag="qsb", name=f"qsb{rep}")
                nc.vector.tensor_copy(q_sb[:], q_ps[:])
                # main path: ys[b][32g+m, h] accumulates y_g = the partial
                # dot over chunks 4j+g; 4 col groups stream concurrently.
                ys = [
                    ppool.tile([128, H], f32, tag=f"y{b}", name=f"y{b}_{rep}")
                    for b in range(BS)
                ]
                zall = spool.tile(
                    [128, BS], mybir.dt.float32r, tag="zall", name=f"zall{rep}"
                )
                for b in range(BS):
                    for half in range(2):
                        i = b * 2 + half
                        xt = xpool.tile(
                            [128, NKT, H], f8, tag="xt", name=f"xt{rep}_{i}"
                        )
                        qs = [nc.sync, nc.scalar]
                        if i == NTILE - 1:
                            # last tile: wave-aligned quarters so only 4
                            # matmuls depend on the final transfer
                            for q in range(4):
                                qs[(upar + q) % 2].dma_start(
                                    xt[:, 4 * q : 4 * q + 4, :],
                                    x_ap[i][:, 4 * q : 4 * q + 4, :],
                                )
                        else:
                            # queue parity tied to body parity: the queue that
                            # stores fin in body u never carries body u+1's
                            # first tile, so the fin-wait can't delay it
                            qs[(upar + i) % 2].dma_start(xt[:], x_ap[i])
                        for jj in range(NKT // NGRP):
                            j = half * (NKT // NGRP) + jj
                            for g in range(NGRP):
                                gk = NGRP * j + g
                                nc.tensor.matmul(
                                    ys[b][32 * g : 32 * g + MREP, :],
                                    lhsT=ccols[:, gk : gk + 1, :],
                                    rhs=xt[:, NGRP * jj + g, :],
                                    start=(j == 0),
                                    stop=(j == NK // NGRP - 1),
                                    tile_position=(0, 32 * g),
                                )
                    # z[p] = sum_h ys[b][p,h] * W[h]/S  (per-sample, overlaps
                    # the next sample's matmuls)
                    tmp = spool.tile([128, H], f32, tag="tmp", name=f"tmp{rep}_{b}")
                    nc.vector.tensor_mul(tmp[:], ys[b][:], wsb[:])
                    with nc.allow_low_precision("f32r y-path dot, ~1e-5 rel"):
                        nc.vector.reduce_sum(
                            zall[:, b : b + 1], tmp[:], axis=mybir.AxisListType.X
                        )

                # out[b] = sum_p zall[p,b]*rowsel[p] + sum_p sel[p,b]*mq2[p]
                o_ps = ppool.tile([1, BS], f32, tag="o", name=f"o{rep}")
                nc.tensor.matmul(
                    o_ps[:], lhsT=rsel[:], rhs=zall[:], start=True, stop=True
                )
                fin = spool.tile([1, BS], f32, tag="fin", name=f"fin{rep}")
                nc.vector.tensor_add(fin[:], o_ps[:], q_sb[:])
                [nc.sync, nc.scalar][upar % 2].dma_start(out_ap[:], fin[:])

            if hw_loop:
                unroll = 4 if hw_loop % 4 == 0 else (2 if hw_loop % 2 == 0 else 1)
                with tc.For_i(0, hw_loop // unroll):
                    for u in range(unroll):
                        emit_body(u, u)
            else:
                for rep in range(repeats):
                    emit_body(rep)

    _legalize_waits(nc)
    return nc


def _prepare_in_maps(x, mask, weight_ema, weight_mean, W, b):
    """Host-side prep: fold the tiny scalar weights into the c vectors
    (float64), quantize x and the scaled c to fp8, shard x/mask over the
    batch dim."""
    import ml_dtypes

    f8 = ml_dtypes.float8_e4m3

    x = np.asarray(x, dtype=np.float32)
    mask = np.asarray(mask)
    weight_ema = np.asarray(weight_ema, dtype=np.float64)
    weight_mean = np.asarray(weight_mean, dtype=np.float64)
    W = np.asarray(W, dtype=np.float64)
    b = np.asarray(b, dtype=np.float64)

    pows = (1.0 - ALPHA) ** np.arange(T - 1, -1, -1, dtype=np.float64)
    wv = ALPHA * pows
    wv[0] = pows[0]
    c = np.float64(weight_ema[0]) * wv + np.float64(weight_mean[0]) / T
    Wsum = float(W.sum())
    c2 = PEN * Wsum * c
    K0 = float(b[0]) - PEN * Wsum * float(c.sum())

    # power-of-two scale putting max|c| ~ 64, well inside fp8e4 normals
    cmax = float(np.abs(c).max())
    S = float(2.0 ** np.floor(np.log2(64.0 / cmax))) if cmax > 0 else 1.0

    # ccols[p, k, m] = S * c[k*128 + p] for every replicated column m
    cq = (c * S).reshape(NK, 128).T.astype(f8)
    ccols = np.ascontiguousarray(np.repeat(cq[:, :, None], MREP, axis=2))
    # c2grid[p, f] = c2[(p % 32) * 128 + f]  (matches mask.reshape(128,128))
    c2grid = np.ascontiguousarray(
        np.tile(c2.reshape(T // 128, 128), (BS, 1)), dtype=np.float32
    )
    sel = np.zeros((128, BS), dtype=np.float32)
    for bb in range(BS):
        sel[bb * (128 // BS) : (bb + 1) * (128 // BS), bb] = 1.0
    w_in = np.ascontiguousarray(
        np.broadcast_to(W.reshape(1, H) / S, (128, H)), dtype=np.float32
    )
    # sel has 32 ones per sample, so K0/32 per partition sums back to K0
    k0_in = np.full((128, 1), K0 / (128 // BS), dtype=np.float32)
    # rowsel: 1.0 at one representative row per column group
    rowsel = np.zeros((128, 1), dtype=np.float32)
    rowsel[[0, 32, 64, 96], 0] = 1.0

    # x tile layout: [b, half, p, k, h] with t = (half*NKT + k)*128 + p
    x8 = x.astype(f8).reshape(B // BS, BS, 2, NKT, 128, H)
    in_maps = []
    for i in range(N_CORES):
        xs = np.ascontiguousarray(x8[i].transpose(0, 1, 3, 2, 4)).reshape(
            NTILE, 128, NKT, H
        )
        ms = np.ascontiguousarray(
            mask[i * BS : (i + 1) * BS].reshape(128, 128).astype(np.uint8)
        )
        in_maps.append(
            {
                "x": xs,
                "mask": ms,
                "ccols": ccols,
                "c2grid": c2grid,
                "sel": sel,
                "w": w_in,
                "k0": k0_in,
                "rowsel": rowsel,
            }
        )
    return in_maps


def _run(inputs, trace=False):
    from concourse.bass_utils import run_bass_kernel_spmd

    if "nc" not in _PROGRAM_CACHE:
        _PROGRAM_CACHE["nc"] = _build_program(repeats=1)
    nc = _PROGRAM_CACHE["nc"]
    in_maps = _prepare_in_maps(**inputs)
    res = run_bass_kernel_spmd(nc, in_maps, list(range(N_CORES)), trace=trace)
    out = np.concatenate(
        [res.results[i]["out"].reshape(BS) for i in range(N_CORES)]
    ).astype(np.float32)
    return out, res


def kernel(**inputs) -> np.ndarray:
    out, _ = _run(inputs, trace=False)
    return out
